# revision 1
# baseline (speedup 1.0000x reference)
"""CrossHazardInteractionLayer TRN2 kernel.

Data-parallel over batch B=8 -> 8 NeuronCores (one batch element each).
Host prep: slice x per core, pre-transpose+cast to bf16 (feature-major),
permute the small per-pair weights, fold the |M|>thr gate structure.
Device per core:
  stage 1 per source s: hT[(t,k), n] = gelu(x[s]^T-major @ W1[s,:] + b1)
    for all 7 targets in one set of bf16 matmuls (fp32 PSUM accumulate),
    exact-erf GELU fused into the PSUM->SBUF copy on the scalar engine,
    bottleneck output packed 2 sources per 128-partition k-tile.
  stage 2 per target t: out[n, d] = x[t] + sum over (s,k) of
    hT * (gate*W2) -- gate folded into W2 at load (DVE broadcast scale),
    b2 handled via constant-ones contraction rows when nonzero,
    residual added from a bf16 copy of x[t], fp32 out.
"""

import os
import numpy as np
import ml_dtypes

import concourse.bass as bass
import concourse.mybir as mybir
import concourse.tile as tile
from concourse import bacc

H = 7
B = 8
S = 2048
D = 768
K = 64
TK = H * K          # 448
P = 128
PASS = 512          # seq rows per pass
NPASS = S // PASS
SUB = PASS // P     # 128-row subchunks per pass (4)
DT = D // P         # d-tiles (6)
THR = 0.05

F32 = mybir.dt.float32
BF16 = mybir.dt.bfloat16
GELU = mybir.ActivationFunctionType.Gelu

_CACHE: dict = {}


def _build(loop_n=None, has_b2=False, act_t=None, act_s=None):
    """act_t[s] = tuple of active targets for source s (packing order);
    act_s[t] = tuple of active sources for target t (packing order)."""
    if act_t is None:
        act_t = tuple(tuple(t for t in range(H) if t != s) for s in range(H))
    if act_s is None:
        act_s = tuple(tuple(s for s in range(H) if s != t) for t in range(H))
    nc = bacc.Bacc("TRN2", target_bir_lowering=False, debug=False)
    xin = nc.declare_dram_parameter("xin", [H, S, D], F32, isOutput=False)
    xtt = nc.declare_dram_parameter("xtt", [H, NPASS, P, DT, PASS], BF16, isOutput=False)
    w1t = nc.declare_dram_parameter("w1t", [H, D, 6 * K], F32, isOutput=False)
    w2t = nc.declare_dram_parameter("w2t", [H, 4 * P, D], F32, isOutput=False)
    b1p = nc.declare_dram_parameter("b1p", [P, 3, H], F32, isOutput=False)
    b2t = nc.declare_dram_parameter("b2t", [H, H, D], F32, isOutput=False)
    gsp = nc.declare_dram_parameter("gsp", [P, 4, H], F32, isOutput=False)
    g7 = nc.declare_dram_parameter("g7", [H, H], F32, isOutput=False)
    out = nc.declare_dram_parameter("out", [H, S, D], F32, isOutput=True)

    import contextlib

    with tile.TileContext(nc) as tc:
        with contextlib.ExitStack() as _loop_ctx:
            if loop_n is not None:
                _loop_ctx.enter_context(tc.For_i(0, loop_n, 1))
            _emit_body(nc, tc, xin, xtt, w1t, w2t, b1p, b2t, gsp, g7, out,
                       has_b2, act_t, act_s)
    nc.compile()
    return nc


def _emit_body(nc, tc, xin, xtt, w1t, w2t, b1p, b2t, gsp, g7, out,
               has_b2, act_t, act_s):
    import math
    # stage-2 contraction rows per target: 64 per active source (+7 ones rows)
    s2rows = [64 * len(act_s[t]) + (H if has_b2 else 0) for t in range(H)]
    s2tiles = [math.ceil(r / P) for r in s2rows]

    with tc.tile_pool(name="static", bufs=1) as st, \
         tc.tile_pool(name="xt", bufs=2) as xtp, \
         tc.tile_pool(name="xnr", bufs=2) as xnp, \
         tc.tile_pool(name="ht", bufs=2) as htp, \
         tc.tile_pool(name="osb", bufs=2) as osp, \
         tc.tile_pool(name="s1_ps", bufs=4, space="PSUM") as s1p, \
         tc.tile_pool(name="s2_ps", bufs=2, space="PSUM") as s2p:

        ring = [nc.sync, nc.scalar]

        # ---- static setup ----
        b1sb = st.tile([P, 3, H], F32, tag="b1sb")
        nc.sync.dma_start(b1sb[:], b1p[:])
        gsb = st.tile([P, 4, H], F32, tag="gsb")
        nc.scalar.dma_start(gsb[:], gsp[:])

        # W1: cast-DMA fp32 -> bf16, [768, 6K] -> [128, 6, 6K]
        w1sb = []
        for s in range(H):
            w = st.tile([P, DT, 6 * K], BF16, tag=f"w1_{s}")
            nc.gpsimd.dma_start(w[:], w1t[s].rearrange("(o p) c -> p o c", p=P))
            w1sb.append(w)

        # W2: gate-scaled bf16 [128, 4, 768] per target (one DMA + one DVE op)
        with tc.tile_pool(name="wstg", bufs=2) as wsp:
            if has_b2:
                g7sb = st.tile([P, H], F32, tag="g7sb")
                nc.sync.dma_start(g7sb[0:H, :], g7[:, :])
                nc.sync.dma_start(g7sb[K:K + H, :], g7[:, :])
            w2sb = []
            for t in range(H):
                w = st.tile([P, 4, D], BF16, tag=f"w2_{t}")
                stg = wsp.tile([P, 4, D], F32, tag="wstg")
                ring[t % 2].dma_start(stg[:], w2t[t].rearrange("(j p) d -> p j d", p=P))
                nc.vector.tensor_tensor(
                    w[:], stg[:], gsb[:, :, t, None].to_broadcast((P, 4, D)),
                    mybir.AluOpType.mult)
                if has_b2:
                    # gate-scaled b2 rows at contraction rows [64*ns, 64*ns+7)
                    r = 64 * len(act_s[t])
                    jb, rb = r // P, r % P
                    bstg = wsp.tile([P, D], F32, tag="bstg")
                    nc.sync.dma_start(bstg[rb:rb + H, :],
                                      b2t[t, 0:H, :])
                    nc.vector.tensor_scalar_mul(
                        w[rb:rb + H, jb, :], bstg[rb:rb + H, :],
                        g7sb[rb:rb + H, t:t + 1])
                w2sb.append(w)

        # ---- passes over sequence ----
        for p in range(NPASS):
            r0 = p * PASS
            hts = []
            for t in range(H):
                ht = htp.tile([P, 4, PASS], BF16, tag=f"ht{t}")
                hts.append(ht)
                if has_b2:
                    r = 64 * len(act_s[t])
                    jb, rb = r // P, r % P
                    nc.vector.memset(ht[rb:P, jb, :], 0.0)
                    nc.vector.memset(ht[rb:rb + H, jb, :], 1.0)

            # stage 1 per source
            for s in range(H):
                nt = len(act_t[s])
                if nt == 0:
                    continue
                xt = xtp.tile([P, DT, PASS], BF16, tag="xt")
                ring[s % 2].dma_start(xt[:], xtt[s, p])
                mchunks = math.ceil(nt * K / P)
                for mc in range(mchunks):
                    msz = min(P, nt * K - mc * P)
                    ps1 = s1p.tile([P, PASS], F32, tag="ps1")
                    for d in range(DT):
                        nc.tensor.matmul(
                            ps1[:msz, :],
                            w1sb[s][:, d, mc * P:mc * P + msz],
                            xt[:, d, :],
                            start=(d == 0), stop=(d == DT - 1))
                    for half in range(msz // K):
                        t = act_t[s][2 * mc + half]
                        q = act_s[t].index(s)
                        nc.scalar.activation(
                            hts[t][(q % 2) * K:(q % 2) * K + K, q // 2, :],
                            ps1[half * K:half * K + K, :],
                            GELU,
                            bias=b1sb[half * K:half * K + K, mc, s:s + 1])

            # stage 2 per target
            for t in range(H):
                xnr = xnp.tile([P, SUB, D], BF16, tag="xnr")
                nc.gpsimd.dma_start(
                    xnr[:], xin[t, r0:r0 + PASS, :].rearrange("(o p) d -> p o d", p=P))
                if s2tiles[t] == 0:
                    # no active sources, no bias: out = x exactly
                    nc.sync.dma_start(out[t, r0:r0 + PASS, :], xin[t, r0:r0 + PASS, :])
                    continue
                osb = osp.tile([P, SUB, D], F32, tag="osb")
                for sc in range(SUB):
                    ps2 = s2p.tile([P, 2, 512], F32, tag="ps2")
                    for n in range(2):
                        for j in range(s2tiles[t]):
                            ksz = min(P, s2rows[t] - j * P) if not has_b2 else P
                            nc.tensor.matmul(
                                ps2[:, n, 0:384],
                                hts[t][0:ksz, j, sc * P:(sc + 1) * P],
                                w2sb[t][0:ksz, j, n * 384:(n + 1) * 384],
                                start=(j == 0), stop=(j == s2tiles[t] - 1))
                    nc.vector.tensor_add(
                        osb[:, sc, :].rearrange("p (a b) -> p a b", a=2),
                        ps2[:, :, 0:384],
                        xnr[:, sc, :].rearrange("p (a b) -> p a b", a=2))
                ring[t % 2].dma_start(
                    out[t, r0:r0 + PASS, :].rearrange("(o p) d -> p o d", p=P),
                    osb[:])


def prepare(inputs):
    """Host prep: gate fold + layout permutes. Returns (in_maps, build_key)."""
    x = np.asarray(inputs["x"], dtype=np.float32)
    M = np.asarray(inputs["M"], dtype=np.float32)
    W1 = np.asarray(inputs["W1"], dtype=np.float32)
    b1 = np.asarray(inputs["b1"], dtype=np.float32)
    W2 = np.asarray(inputs["W2"], dtype=np.float32)
    b2 = np.asarray(inputs["b2"], dtype=np.float32)

    eye = np.eye(H, dtype=bool)
    gate = np.where((np.abs(M) > THR) & (~eye), M, np.zeros_like(M)).astype(np.float32)
    has_b2 = bool(np.any(b2))
    act = gate != 0.0
    act_t = tuple(tuple(int(t) for t in range(H) if act[s, t]) for s in range(H))
    act_s = tuple(tuple(int(s) for s in range(H) if act[s, t]) for t in range(H))

    # W1 columns packed per source in act_t order: [H, D, 6K]
    w1t = np.zeros((H, D, 6 * K), np.float32)
    b1f = np.zeros((H, 3 * P), np.float32)
    for s in range(H):
        for i, t in enumerate(act_t[s]):
            w1t[s, :, i * K:(i + 1) * K] = W1[s, t]
            b1f[s, i * K:(i + 1) * K] = b1[s, t]
    b1p = np.ascontiguousarray(b1f.reshape(H, 3, P).transpose(2, 1, 0))

    # W2 rows packed per target in act_s order: [H, 4P, D]; gate expansion [P,4,H]
    w2f = np.zeros((H, 4 * P, D), np.float32)
    gsf = np.zeros((H, 4 * P), np.float32)
    for t in range(H):
        for q, s in enumerate(act_s[t]):
            w2f[t, q * K:(q + 1) * K, :] = W2[s, t]
            gsf[t, q * K:(q + 1) * K] = gate[s, t]
        if has_b2:
            r = K * len(act_s[t])
            gsf[t, r:r + H] = 1.0  # bias rows get scaled separately
    gsp = np.ascontiguousarray(gsf.reshape(H, 4, P).transpose(2, 1, 0))
    # b2 rows per target in act_s order
    b2t = np.zeros((H, H, D), np.float32)
    for t in range(H):
        for q, s in enumerate(act_s[t]):
            b2t[t, q] = b2[s, t]

    in_maps = []
    for b in range(B):
        xb = np.ascontiguousarray(x[:, b])
        xbf = xb.astype(ml_dtypes.bfloat16)
        # [s, q(pass), p, o, n]: element = xbf[s, q*PASS+n, o*P+p]
        xtb = np.ascontiguousarray(
            xbf.reshape(H, NPASS, PASS, DT, P).transpose(0, 1, 4, 3, 2))
        in_maps.append({
            "xin": xb, "xtt": xtb,
            "w1t": w1t, "w2t": w2f, "b1p": b1p, "b2t": b2t,
            "gsp": gsp, "g7": gate,
        })
    return in_maps, (has_b2, act_t, act_s)


def kernel(**inputs):
    in_maps, key = prepare(inputs)
    runner = _get_runner(key)
    outs = runner.run(in_maps)
    return np.stack([outs[b]["out"] for b in range(B)], axis=1)


class _Runner:
    """Cached PJRT executor for the SPMD bass kernel (8 cores, no donation)."""

    def __init__(self, nc):
        import jax
        from jax.sharding import Mesh, PartitionSpec, NamedSharding
        from jax.experimental.shard_map import shard_map
        from concourse import bass2jax
        bass2jax.install_neuronx_cc_hook()

        self.jax = jax
        part_name = nc.partition_id_tensor.name if nc.partition_id_tensor else None
        in_names, out_names, out_avals, zero_shapes = [], [], [], []
        for alloc in nc.m.functions[0].allocations:
            if not isinstance(alloc, mybir.MemoryLocationSet):
                continue
            name = alloc.memorylocations[0].name
            if alloc.kind == "ExternalInput":
                if name != part_name:
                    in_names.append(name)
            elif alloc.kind == "ExternalOutput":
                out_names.append(name)
                shape = tuple(alloc.tensor_shape)
                dtype = mybir.dt.np(alloc.dtype)
                out_avals.append(jax.core.ShapedArray(shape, dtype))
                zero_shapes.append((shape, dtype))
        self.n_params = len(in_names)
        self.in_names = list(in_names)
        self.out_names = out_names
        self.out_avals = out_avals
        self.zero_shapes = zero_shapes
        bind_names = tuple(in_names) + tuple(out_names)
        if part_name is not None:
            bind_names = bind_names + (part_name,)

        def _body(*args):
            operands = list(args)
            if part_name is not None:
                operands.append(bass2jax.partition_id_tensor())
            outs = bass2jax._bass_exec_p.bind(
                *operands,
                out_avals=tuple(out_avals),
                in_names=bind_names,
                out_names=tuple(out_names),
                lowering_input_output_aliases=(),
                sim_require_finite=True,
                sim_require_nnan=True,
                nc=nc,
            )
            return tuple(outs)

        devices = jax.devices()[:B]
        self.mesh = Mesh(np.asarray(devices), ("core",))
        spec = PartitionSpec("core")
        self.sharding = NamedSharding(self.mesh, spec)
        n_in = self.n_params + len(out_names)
        self.fn = jax.jit(
            shard_map(_body, mesh=self.mesh,
                      in_specs=(spec,) * n_in,
                      out_specs=(spec,) * len(out_names),
                      check_rep=False),
            keep_unused=True,
        )

    def _concat_args(self, in_maps):
        args = []
        for i, name in enumerate(self.in_names):
            args.append(np.concatenate([np.asarray(m[name]) for m in in_maps], axis=0))
        for shape, dtype in self.zero_shapes:
            args.append(np.zeros((B * shape[0],) + shape[1:], dtype))
        return args

    def run(self, in_maps):
        out_arrs = self.fn(*self._concat_args(in_maps))
        res = []
        for c in range(B):
            d = {}
            for i, name in enumerate(self.out_names):
                shape = self.out_avals[i].shape
                d[name] = np.asarray(out_arrs[i]).reshape((B,) + shape)[c]
            res.append(d)
        return res

    def benchmark(self, in_maps, iters=10):
        jax = self.jax
        args = [jax.device_put(a, self.sharding) for a in self._concat_args(in_maps)]
        outs = self.fn(*args)  # warmup / compile
        jax.block_until_ready(outs)
        import time
        t0 = time.perf_counter()
        for _ in range(iters):
            outs = self.fn(*args)
        jax.block_until_ready(outs)
        t1 = time.perf_counter()
        return (t1 - t0) / iters


def _get_runner(key) -> _Runner:
    has_b2, act_t, act_s = key
    ck = ("runner", key)
    if ck not in _CACHE:
        _CACHE[ck] = _Runner(_build(has_b2=has_b2, act_t=act_t, act_s=act_s))
    return _CACHE[ck]



# revision 3
# speedup vs baseline: 1.5613x; 1.5613x over previous
"""CrossHazardInteractionLayer TRN2 kernel (v2: HBM-traffic-minimized).

Data-parallel over batch B=8 -> 8 NeuronCores (one batch element each).
Host prep: fold the |M|>thr gate into W2 (pre-scaled), cast W1/W2/x to
bf16, transpose x to feature-major (d on partitions) once.  Device:
  stage 1 per source s: hT[(t,k), n] = gelu(x[s]^T-tiles @ W1[s,:]) for
    all active targets, packed 2 targets per 128-col chunk; exact-erf
    GELU fused into the PSUM->SBUF copy on the scalar engine.
  stage 2 per target t: outT[d, n] = x[t]^T + sum over (s,k) j-tiles of
    (gate*W2)^T-stationary @ hT-moving -- output stays d-major so the
    SAME x^T tile loaded for stage 1 provides the residual (no second
    x load), fused into the PSUM->SBUF add on the vector engine (bf16
    out).  Output is written d-major bf16; host un-transposes + upcasts.
HBM traffic/core: 22 MB x + 8.3 MB weights + 22 MB out = ~52 MB
(baseline moved ~103 MB), so the kernel is tensor-engine-bound.
"""

import numpy as np
import ml_dtypes

import concourse.bass as bass
import concourse.mybir as mybir
import concourse.tile as tile
from concourse import bacc

H = 7
B = 8
S = 2048
D = 768
K = 64
P = 128
PASS = 512          # seq cols per pass
NPASS = S // PASS
DT = D // P         # d-tiles (6)
THR = 0.05

F32 = mybir.dt.float32
BF16 = mybir.dt.bfloat16
GELU = mybir.ActivationFunctionType.Gelu

_CACHE: dict = {}


def _build(has_b2=False, act_t=None, act_s=None):
    """act_t[s] = tuple of active targets for source s (packing order);
    act_s[t] = tuple of active sources for target t (packing order)."""
    import math
    if act_t is None:
        act_t = tuple(tuple(t for t in range(H) if t != s) for s in range(H))
    if act_s is None:
        act_s = tuple(tuple(s for s in range(H) if s != t) for t in range(H))
    mchunks = [math.ceil(len(act_t[s]) * K / P) for s in range(H)]
    s2tiles = [math.ceil(len(act_s[t]) * K / P) for t in range(H)]

    nc = bacc.Bacc("TRN2", target_bir_lowering=False, debug=False)
    xtd = nc.declare_dram_parameter("xtd", [H, NPASS, P, DT, PASS], BF16, isOutput=False)
    w1h = nc.declare_dram_parameter("w1h", [H, P, DT, 3 * P], BF16, isOutput=False)
    w2h = nc.declare_dram_parameter("w2h", [H, P, 3, D], BF16, isOutput=False)
    b1h = nc.declare_dram_parameter("b1h", [P, 3, H], F32, isOutput=False)
    b2h = nc.declare_dram_parameter("b2h", [P, DT, H], F32, isOutput=False)
    outd = nc.declare_dram_parameter("outd", [H, NPASS, P, DT, PASS], BF16, isOutput=True)

    with tile.TileContext(nc) as tc:
        with tc.tile_pool(name="static", bufs=1) as st, \
             tc.tile_pool(name="xt", bufs=2) as xtp, \
             tc.tile_pool(name="ht", bufs=2) as htp, \
             tc.tile_pool(name="osb", bufs=2) as osp, \
             tc.tile_pool(name="s1_ps", bufs=4, space="PSUM") as s1p, \
             tc.tile_pool(name="s2_ps", bufs=4, space="PSUM") as s2p:

            # ---- static setup (weights resident in SBUF, bf16) ----
            b1sb = st.tile([P, 3, H], F32, tag="b1sb")
            nc.sync.dma_start(b1sb[:], b1h[:])
            if has_b2:
                b2sb = st.tile([P, DT, H], F32, tag="b2sb")
                nc.gpsimd.dma_start(b2sb[:], b2h[:])
            w1sb = []
            for s in range(H):
                w = st.tile([P, DT, 3 * P], BF16, tag=f"w1_{s}")
                nc.sync.dma_start(w[:], w1h[s])
                w1sb.append(w)
            w2sb = []
            for t in range(H):
                jt = max(s2tiles[t], 1)
                w = st.tile([P, jt, D], BF16, tag=f"w2_{t}")
                nc.gpsimd.dma_start(w[:], w2h[t, :, 0:jt, :])
                w2sb.append(w)

            # ---- passes over the sequence ----
            for p in range(NPASS):
                xts = []
                for s in range(H):
                    xt = xtp.tile([P, DT, PASS], BF16, tag=f"xt{s}")
                    nc.sync.dma_start(xt[:], xtd[s, p])
                    xts.append(xt)
                hts = [htp.tile([P, 3, PASS], BF16, tag=f"ht{t}", name=f"ht{t}")
                       for t in range(H)]

                # stage 1: hT = gelu(x^T-major contractions with W1)
                for s in range(H):
                    nt = len(act_t[s])
                    for mc in range(mchunks[s]):
                        msz = min(P, nt * K - mc * P)
                        ps1 = s1p.tile([P, PASS], F32, tag="ps1")
                        for d in range(DT):
                            nc.tensor.matmul(
                                ps1[:msz, :],
                                w1sb[s][:, d, mc * P:mc * P + msz],
                                xts[s][:, d, :],
                                start=(d == 0), stop=(d == DT - 1))
                        for half in range(msz // K):
                            t = act_t[s][2 * mc + half]
                            q = act_s[t].index(s)
                            nc.scalar.activation(
                                hts[t][(q % 2) * K:(q % 2) * K + K, q // 2, :],
                                ps1[half * K:half * K + K, :],
                                GELU,
                                bias=b1sb[half * K:half * K + K, mc, s:s + 1])

                # stage 2: outT[d,n] = x^T + (gate*W2)^T @ hT, d-major
                for t in range(H):
                    ns = len(act_s[t])
                    for oh in range(2):          # store in 3-dtile halves
                        osb = osp.tile([P, 3, PASS], BF16, tag="osb")
                        for oi in range(3):
                            o = oh * 3 + oi
                            if ns == 0:
                                nc.vector.tensor_copy(osb[:, oi, :], xts[t][:, o, :])
                            else:
                                ps2 = s2p.tile([P, PASS], F32, tag="ps2")
                                for j in range(s2tiles[t]):
                                    ksz = min(P, ns * K - j * P)
                                    nc.tensor.matmul(
                                        ps2[:, :],
                                        w2sb[t][0:ksz, j, o * P:(o + 1) * P],
                                        hts[t][0:ksz, j, :],
                                        start=(j == 0), stop=(j == s2tiles[t] - 1))
                                nc.vector.tensor_add(osb[:, oi, :], ps2[:, :], xts[t][:, o, :])
                            if has_b2:
                                nc.vector.tensor_scalar_add(
                                    osb[:, oi, :], osb[:, oi, :], b2sb[:, o, t:t + 1])
                        nc.scalar.dma_start(outd[t, p, :, oh * 3:oh * 3 + 3, :], osb[:])
    nc.compile()
    return nc


def prepare(inputs):
    """Host prep: gate fold + bf16 cast + layout permutes.
    Returns (in_maps, build_key)."""
    x = np.asarray(inputs["x"], dtype=np.float32)
    M = np.asarray(inputs["M"], dtype=np.float32)
    W1 = np.asarray(inputs["W1"], dtype=np.float32)
    b1 = np.asarray(inputs["b1"], dtype=np.float32)
    W2 = np.asarray(inputs["W2"], dtype=np.float32)
    b2 = np.asarray(inputs["b2"], dtype=np.float32)

    eye = np.eye(H, dtype=bool)
    gate = np.where((np.abs(M) > THR) & (~eye), M, np.zeros_like(M)).astype(np.float32)
    has_b2 = bool(np.any(b2))
    act = gate != 0.0
    act_t = tuple(tuple(int(t) for t in range(H) if act[s, t]) for s in range(H))
    act_s = tuple(tuple(int(s) for s in range(H) if act[s, t]) for t in range(H))

    # W1 columns packed per source in act_t order -> [H, 128, 6, 384] bf16
    w1f = np.zeros((H, D, 3 * P), np.float32)
    b1f = np.zeros((H, 3 * P), np.float32)
    for s in range(H):
        for i, t in enumerate(act_t[s]):
            w1f[s, :, i * K:(i + 1) * K] = W1[s, t]
            b1f[s, i * K:(i + 1) * K] = b1[s, t]
    w1h = np.ascontiguousarray(
        w1f.reshape(H, DT, P, 3 * P).transpose(0, 2, 1, 3)).astype(ml_dtypes.bfloat16)
    b1h = np.ascontiguousarray(b1f.reshape(H, 3, P).transpose(2, 1, 0))

    # gate-scaled W2 rows packed per target in act_s order -> [H, 128, 3, 768]
    w2f = np.zeros((H, 3 * P, D), np.float32)
    for t in range(H):
        for q, s in enumerate(act_s[t]):
            w2f[t, q * K:(q + 1) * K, :] = gate[s, t] * W2[s, t]
    w2h = np.ascontiguousarray(
        w2f.reshape(H, 3, P, D).transpose(0, 2, 1, 3)).astype(ml_dtypes.bfloat16)

    # gate-folded b2 per target: [128, 6, 7]
    b2f = np.einsum("st,std->td", gate, b2).astype(np.float32)   # [H, D]
    b2h = np.ascontiguousarray(b2f.reshape(H, DT, P).transpose(2, 1, 0))

    in_maps = []
    for b in range(B):
        xbf = x[:, b].astype(ml_dtypes.bfloat16)
        # [s, q, p, o, n]: element = xbf[s, q*PASS+n, o*P+p]
        xtb = np.ascontiguousarray(
            xbf.reshape(H, NPASS, PASS, DT, P).transpose(0, 1, 4, 3, 2))
        in_maps.append({
            "xtd": xtb, "w1h": w1h, "w2h": w2h, "b1h": b1h, "b2h": b2h,
        })
    return in_maps, (has_b2, act_t, act_s)


def assemble(outs):
    """Per-core outd [H, NPASS, 128, 6, 512] bf16 -> full [H, B, S, D] f32."""
    res = []
    for b in range(B):
        o = np.asarray(outs[b]["outd"])
        # out[t, q*512+n, o*128+p] = outd[t, q, p, o, n]
        res.append(o.transpose(0, 1, 4, 3, 2).reshape(H, S, D).astype(np.float32))
    return np.stack(res, axis=1)


def kernel(**inputs):
    in_maps, key = prepare(inputs)
    runner = _get_runner(key)
    outs = runner.run(in_maps)
    return assemble(outs)


class _Runner:
    """Cached PJRT executor for the SPMD bass kernel (8 cores, no donation)."""

    def __init__(self, nc):
        import jax
        from jax.sharding import Mesh, PartitionSpec, NamedSharding
        from jax.experimental.shard_map import shard_map
        from concourse import bass2jax
        bass2jax.install_neuronx_cc_hook()

        self.jax = jax
        part_name = nc.partition_id_tensor.name if nc.partition_id_tensor else None
        in_names, out_names, out_avals, zero_shapes = [], [], [], []
        for alloc in nc.m.functions[0].allocations:
            if not isinstance(alloc, mybir.MemoryLocationSet):
                continue
            name = alloc.memorylocations[0].name
            if alloc.kind == "ExternalInput":
                if name != part_name:
                    in_names.append(name)
            elif alloc.kind == "ExternalOutput":
                out_names.append(name)
                shape = tuple(alloc.tensor_shape)
                dtype = mybir.dt.np(alloc.dtype)
                out_avals.append(jax.core.ShapedArray(shape, dtype))
                zero_shapes.append((shape, dtype))
        self.n_params = len(in_names)
        self.in_names = list(in_names)
        self.out_names = out_names
        self.out_avals = out_avals
        self.zero_shapes = zero_shapes
        bind_names = tuple(in_names) + tuple(out_names)
        if part_name is not None:
            bind_names = bind_names + (part_name,)

        def _body(*args):
            operands = list(args)
            if part_name is not None:
                operands.append(bass2jax.partition_id_tensor())
            outs = bass2jax._bass_exec_p.bind(
                *operands,
                out_avals=tuple(out_avals),
                in_names=bind_names,
                out_names=tuple(out_names),
                lowering_input_output_aliases=(),
                sim_require_finite=True,
                sim_require_nnan=True,
                nc=nc,
            )
            return tuple(outs)

        devices = jax.devices()[:B]
        self.mesh = Mesh(np.asarray(devices), ("core",))
        spec = PartitionSpec("core")
        self.sharding = NamedSharding(self.mesh, spec)
        n_in = self.n_params + len(out_names)
        self.fn = jax.jit(
            shard_map(_body, mesh=self.mesh,
                      in_specs=(spec,) * n_in,
                      out_specs=(spec,) * len(out_names),
                      check_rep=False),
            keep_unused=True,
        )

    def _concat_args(self, in_maps):
        args = []
        for i, name in enumerate(self.in_names):
            args.append(np.concatenate([np.asarray(m[name]) for m in in_maps], axis=0))
        for shape, dtype in self.zero_shapes:
            args.append(np.zeros((B * shape[0],) + shape[1:], dtype))
        return args

    def run(self, in_maps):
        out_arrs = self.fn(*self._concat_args(in_maps))
        res = []
        for c in range(B):
            d = {}
            for i, name in enumerate(self.out_names):
                shape = self.out_avals[i].shape
                d[name] = np.asarray(out_arrs[i]).reshape((B,) + shape)[c]
            res.append(d)
        return res

    def benchmark(self, in_maps, iters=10):
        jax = self.jax
        args = [jax.device_put(a, self.sharding) for a in self._concat_args(in_maps)]
        outs = self.fn(*args)  # warmup / compile
        jax.block_until_ready(outs)
        import time
        t0 = time.perf_counter()
        for _ in range(iters):
            outs = self.fn(*args)
        jax.block_until_ready(outs)
        t1 = time.perf_counter()
        return (t1 - t0) / iters


def _build_from_key(key):
    has_b2, act_t, act_s = key
    return _build(has_b2=has_b2, act_t=act_t, act_s=act_s)


def _get_runner(key) -> _Runner:
    ck = ("runner", key)
    if ck not in _CACHE:
        _CACHE[ck] = _Runner(_build_from_key(key))
    return _CACHE[ck]


# revision 5
# speedup vs baseline: 1.5640x; 1.0017x over previous
"""CrossHazardInteractionLayer TRN2 kernel (v2: HBM-traffic-minimized).

Data-parallel over batch B=8 -> 8 NeuronCores (one batch element each).
Host prep: fold the |M|>thr gate into W2 (pre-scaled), cast W1/W2/x to
bf16, transpose x to feature-major (d on partitions) once.  Device:
  stage 1 per source s: hT[(t,k), n] = gelu(x[s]^T-tiles @ W1[s,:]) for
    all active targets, packed 2 targets per 128-col chunk; exact-erf
    GELU fused into the PSUM->SBUF copy on the scalar engine.
  stage 2 per target t: outT[d, n] = x[t]^T + sum over (s,k) j-tiles of
    (gate*W2)^T-stationary @ hT-moving -- output stays d-major so the
    SAME x^T tile loaded for stage 1 provides the residual (no second
    x load), fused into the PSUM->SBUF add on the vector engine (bf16
    out).  Output is written d-major bf16; host un-transposes + upcasts.
HBM traffic/core: 22 MB x + 8.3 MB weights + 22 MB out = ~52 MB
(baseline moved ~103 MB), so the kernel is tensor-engine-bound.
"""

import numpy as np
import ml_dtypes

import concourse.bass as bass
import concourse.mybir as mybir
import concourse.tile as tile
from concourse import bacc

H = 7
B = 8
S = 2048
D = 768
K = 64
P = 128
PASS = 512          # seq cols per pass
NPASS = S // PASS
DT = D // P         # d-tiles (6)
THR = 0.05

F32 = mybir.dt.float32
BF16 = mybir.dt.bfloat16
GELU = mybir.ActivationFunctionType.Gelu

_CACHE: dict = {}


def _build(has_b2=False, act_t=None, act_s=None):
    """act_t[s] = tuple of active targets for source s (packing order);
    act_s[t] = tuple of active sources for target t (packing order)."""
    import math
    if act_t is None:
        act_t = tuple(tuple(t for t in range(H) if t != s) for s in range(H))
    if act_s is None:
        act_s = tuple(tuple(s for s in range(H) if s != t) for t in range(H))
    mchunks = [math.ceil(len(act_t[s]) * K / P) for s in range(H)]
    s2tiles = [math.ceil(len(act_s[t]) * K / P) for t in range(H)]

    nc = bacc.Bacc("TRN2", target_bir_lowering=False, debug=False)
    xtd = nc.declare_dram_parameter("xtd", [H, NPASS, P, DT, PASS], BF16, isOutput=False)
    w1h = nc.declare_dram_parameter("w1h", [H, P, DT, 3 * P], BF16, isOutput=False)
    w2h = nc.declare_dram_parameter("w2h", [H, P, 3, D], BF16, isOutput=False)
    b1h = nc.declare_dram_parameter("b1h", [P, 3, H], F32, isOutput=False)
    b2h = nc.declare_dram_parameter("b2h", [P, DT, H], F32, isOutput=False)
    outd = nc.declare_dram_parameter("outd", [H, NPASS, P, DT, PASS], BF16, isOutput=True)

    with tile.TileContext(nc) as tc:
        with tc.tile_pool(name="static", bufs=1) as st, \
             tc.tile_pool(name="xt", bufs=2) as xtp, \
             tc.tile_pool(name="ht", bufs=2) as htp, \
             tc.tile_pool(name="osb", bufs=2) as osp, \
             tc.tile_pool(name="s1_ps", bufs=4, space="PSUM") as s1p, \
             tc.tile_pool(name="s2_ps", bufs=4, space="PSUM") as s2p:

            # ---- static setup ----
            # Loads are split across the two HWDGE queues (sync, scalar) in
            # consumption order, so a consumer's queue-prefix wait covers
            # only tiles it actually needs soon.  w1[s]/xt[s] interleave at
            # pass 0 so stage-1 s=0 starts after ~1.4 MB of DMA, not 10 MB.
            # Stores go on the gpsimd SWDGE queue so they never block GELUs.
            ring = [nc.sync, nc.scalar]
            b1sb = st.tile([P, 3, H], F32, tag="b1sb")
            nc.scalar.dma_start(b1sb[:], b1h[:])
            if has_b2:
                b2sb = st.tile([P, DT, H], F32, tag="b2sb")
                nc.gpsimd.dma_start(b2sb[:], b2h[:])
            w2sb = []
            for t in range(H):
                jt = max(s2tiles[t], 1)
                w = st.tile([P, jt, D], BF16, tag=f"w2_{t}")
                nc.gpsimd.dma_start(w[:], w2h[t, :, 0:jt, :])
                w2sb.append(w)
            w1sb = [None] * H

            # ---- passes over the sequence ----
            for p in range(NPASS):
                xts = []
                for s in range(H):
                    if p == 0:
                        w = st.tile([P, DT, 3 * P], BF16, tag=f"w1_{s}",
                                    name=f"w1_{s}")
                        ring[s % 2].dma_start(w[:], w1h[s])
                        w1sb[s] = w
                    xt = xtp.tile([P, DT, PASS], BF16, tag=f"xt{s}")
                    ring[s % 2].dma_start(xt[:], xtd[s, p])
                    xts.append(xt)
                hts = [htp.tile([P, 3, PASS], BF16, tag=f"ht{t}", name=f"ht{t}")
                       for t in range(H)]

                # stage 1: hT = gelu(x^T-major contractions with W1)
                for s in range(H):
                    nt = len(act_t[s])
                    for mc in range(mchunks[s]):
                        msz = min(P, nt * K - mc * P)
                        ps1 = s1p.tile([P, PASS], F32, tag="ps1")
                        for d in range(DT):
                            nc.tensor.matmul(
                                ps1[:msz, :],
                                w1sb[s][:, d, mc * P:mc * P + msz],
                                xts[s][:, d, :],
                                start=(d == 0), stop=(d == DT - 1))
                        for half in range(msz // K):
                            t = act_t[s][2 * mc + half]
                            q = act_s[t].index(s)
                            nc.scalar.activation(
                                hts[t][(q % 2) * K:(q % 2) * K + K, q // 2, :],
                                ps1[half * K:half * K + K, :],
                                GELU,
                                bias=b1sb[half * K:half * K + K, mc, s:s + 1])

                # stage 2: outT[d,n] = x^T + (gate*W2)^T @ hT, d-major
                for t in range(H):
                    ns = len(act_s[t])
                    for oh in range(2):          # store in 3-dtile halves
                        osb = osp.tile([P, 3, PASS], BF16, tag="osb")
                        for oi in range(3):
                            o = oh * 3 + oi
                            if ns == 0:
                                nc.vector.tensor_copy(osb[:, oi, :], xts[t][:, o, :])
                            else:
                                ps2 = s2p.tile([P, PASS], F32, tag="ps2")
                                for j in range(s2tiles[t]):
                                    ksz = min(P, ns * K - j * P)
                                    nc.tensor.matmul(
                                        ps2[:, :],
                                        w2sb[t][0:ksz, j, o * P:(o + 1) * P],
                                        hts[t][0:ksz, j, :],
                                        start=(j == 0), stop=(j == s2tiles[t] - 1))
                                nc.vector.tensor_add(osb[:, oi, :], ps2[:, :], xts[t][:, o, :])
                            if has_b2:
                                nc.vector.tensor_scalar_add(
                                    osb[:, oi, :], osb[:, oi, :], b2sb[:, o, t:t + 1])
                        nc.gpsimd.dma_start(outd[t, p, :, oh * 3:oh * 3 + 3, :], osb[:])
    nc.compile()
    return nc


def prepare(inputs):
    """Host prep: gate fold + bf16 cast + layout permutes.
    Returns (in_maps, build_key)."""
    x = np.asarray(inputs["x"], dtype=np.float32)
    M = np.asarray(inputs["M"], dtype=np.float32)
    W1 = np.asarray(inputs["W1"], dtype=np.float32)
    b1 = np.asarray(inputs["b1"], dtype=np.float32)
    W2 = np.asarray(inputs["W2"], dtype=np.float32)
    b2 = np.asarray(inputs["b2"], dtype=np.float32)

    eye = np.eye(H, dtype=bool)
    gate = np.where((np.abs(M) > THR) & (~eye), M, np.zeros_like(M)).astype(np.float32)
    has_b2 = bool(np.any(b2))
    act = gate != 0.0
    act_t = tuple(tuple(int(t) for t in range(H) if act[s, t]) for s in range(H))
    act_s = tuple(tuple(int(s) for s in range(H) if act[s, t]) for t in range(H))

    # W1 columns packed per source in act_t order -> [H, 128, 6, 384] bf16
    w1f = np.zeros((H, D, 3 * P), np.float32)
    b1f = np.zeros((H, 3 * P), np.float32)
    for s in range(H):
        for i, t in enumerate(act_t[s]):
            w1f[s, :, i * K:(i + 1) * K] = W1[s, t]
            b1f[s, i * K:(i + 1) * K] = b1[s, t]
    w1h = np.ascontiguousarray(
        w1f.reshape(H, DT, P, 3 * P).transpose(0, 2, 1, 3)).astype(ml_dtypes.bfloat16)
    b1h = np.ascontiguousarray(b1f.reshape(H, 3, P).transpose(2, 1, 0))

    # gate-scaled W2 rows packed per target in act_s order -> [H, 128, 3, 768]
    w2f = np.zeros((H, 3 * P, D), np.float32)
    for t in range(H):
        for q, s in enumerate(act_s[t]):
            w2f[t, q * K:(q + 1) * K, :] = gate[s, t] * W2[s, t]
    w2h = np.ascontiguousarray(
        w2f.reshape(H, 3, P, D).transpose(0, 2, 1, 3)).astype(ml_dtypes.bfloat16)

    # gate-folded b2 per target: [128, 6, 7]
    b2f = np.einsum("st,std->td", gate, b2).astype(np.float32)   # [H, D]
    b2h = np.ascontiguousarray(b2f.reshape(H, DT, P).transpose(2, 1, 0))

    in_maps = []
    for b in range(B):
        xbf = x[:, b].astype(ml_dtypes.bfloat16)
        # [s, q, p, o, n]: element = xbf[s, q*PASS+n, o*P+p]
        xtb = np.ascontiguousarray(
            xbf.reshape(H, NPASS, PASS, DT, P).transpose(0, 1, 4, 3, 2))
        in_maps.append({
            "xtd": xtb, "w1h": w1h, "w2h": w2h, "b1h": b1h, "b2h": b2h,
        })
    return in_maps, (has_b2, act_t, act_s)


def assemble(outs):
    """Per-core outd [H, NPASS, 128, 6, 512] bf16 -> full [H, B, S, D] f32."""
    res = []
    for b in range(B):
        o = np.asarray(outs[b]["outd"])
        # out[t, q*512+n, o*128+p] = outd[t, q, p, o, n]
        res.append(o.transpose(0, 1, 4, 3, 2).reshape(H, S, D).astype(np.float32))
    return np.stack(res, axis=1)


def kernel(**inputs):
    in_maps, key = prepare(inputs)
    runner = _get_runner(key)
    outs = runner.run(in_maps)
    return assemble(outs)


class _Runner:
    """Cached PJRT executor for the SPMD bass kernel (8 cores, no donation)."""

    def __init__(self, nc):
        import jax
        from jax.sharding import Mesh, PartitionSpec, NamedSharding
        from jax.experimental.shard_map import shard_map
        from concourse import bass2jax
        bass2jax.install_neuronx_cc_hook()

        self.jax = jax
        part_name = nc.partition_id_tensor.name if nc.partition_id_tensor else None
        in_names, out_names, out_avals, zero_shapes = [], [], [], []
        for alloc in nc.m.functions[0].allocations:
            if not isinstance(alloc, mybir.MemoryLocationSet):
                continue
            name = alloc.memorylocations[0].name
            if alloc.kind == "ExternalInput":
                if name != part_name:
                    in_names.append(name)
            elif alloc.kind == "ExternalOutput":
                out_names.append(name)
                shape = tuple(alloc.tensor_shape)
                dtype = mybir.dt.np(alloc.dtype)
                out_avals.append(jax.core.ShapedArray(shape, dtype))
                zero_shapes.append((shape, dtype))
        self.n_params = len(in_names)
        self.in_names = list(in_names)
        self.out_names = out_names
        self.out_avals = out_avals
        self.zero_shapes = zero_shapes
        bind_names = tuple(in_names) + tuple(out_names)
        if part_name is not None:
            bind_names = bind_names + (part_name,)

        def _body(*args):
            operands = list(args)
            if part_name is not None:
                operands.append(bass2jax.partition_id_tensor())
            outs = bass2jax._bass_exec_p.bind(
                *operands,
                out_avals=tuple(out_avals),
                in_names=bind_names,
                out_names=tuple(out_names),
                lowering_input_output_aliases=(),
                sim_require_finite=True,
                sim_require_nnan=True,
                nc=nc,
            )
            return tuple(outs)

        devices = jax.devices()[:B]
        self.mesh = Mesh(np.asarray(devices), ("core",))
        spec = PartitionSpec("core")
        self.sharding = NamedSharding(self.mesh, spec)
        n_in = self.n_params + len(out_names)
        self.fn = jax.jit(
            shard_map(_body, mesh=self.mesh,
                      in_specs=(spec,) * n_in,
                      out_specs=(spec,) * len(out_names),
                      check_rep=False),
            keep_unused=True,
        )

    def _concat_args(self, in_maps):
        args = []
        for i, name in enumerate(self.in_names):
            args.append(np.concatenate([np.asarray(m[name]) for m in in_maps], axis=0))
        for shape, dtype in self.zero_shapes:
            args.append(np.zeros((B * shape[0],) + shape[1:], dtype))
        return args

    def run(self, in_maps):
        out_arrs = self.fn(*self._concat_args(in_maps))
        res = []
        for c in range(B):
            d = {}
            for i, name in enumerate(self.out_names):
                shape = self.out_avals[i].shape
                d[name] = np.asarray(out_arrs[i]).reshape((B,) + shape)[c]
            res.append(d)
        return res

    def benchmark(self, in_maps, iters=10):
        jax = self.jax
        args = [jax.device_put(a, self.sharding) for a in self._concat_args(in_maps)]
        outs = self.fn(*args)  # warmup / compile
        jax.block_until_ready(outs)
        import time
        t0 = time.perf_counter()
        for _ in range(iters):
            outs = self.fn(*args)
        jax.block_until_ready(outs)
        t1 = time.perf_counter()
        return (t1 - t0) / iters


def _build_from_key(key):
    has_b2, act_t, act_s = key
    return _build(has_b2=has_b2, act_t=act_t, act_s=act_s)


def _get_runner(key) -> _Runner:
    ck = ("runner", key)
    if ck not in _CACHE:
        _CACHE[ck] = _Runner(_build_from_key(key))
    return _CACHE[ck]


# revision 8
# speedup vs baseline: 1.7228x; 1.1015x over previous
"""CrossHazardInteractionLayer TRN2 kernel (v2: HBM-traffic-minimized).

Data-parallel over batch B=8 -> 8 NeuronCores (one batch element each).
Host prep: fold the |M|>thr gate into W2 (pre-scaled), cast W1/W2/x to
bf16, transpose x to feature-major (d on partitions) once.  Device:
  stage 1 per source s: hT[(t,k), n] = gelu(x[s]^T-tiles @ W1[s,:]) for
    all active targets, packed 2 targets per 128-col chunk; exact-erf
    GELU fused into the PSUM->SBUF copy on the scalar engine.
  stage 2 per target t: outT[d, n] = x[t]^T + sum over (s,k) j-tiles of
    (gate*W2)^T-stationary @ hT-moving -- output stays d-major so the
    SAME x^T tile loaded for stage 1 provides the residual (no second
    x load), fused into the PSUM->SBUF add on the vector engine (bf16
    out).  Output is written d-major bf16; host un-transposes + upcasts.
HBM traffic/core: 22 MB x + 8.3 MB weights + 22 MB out = ~52 MB
(baseline moved ~103 MB), so the kernel is tensor-engine-bound.
"""

import numpy as np
import ml_dtypes

import concourse.bass as bass
import concourse.mybir as mybir
import concourse.tile as tile
from concourse import bacc

H = 7
B = 8
S = 2048
D = 768
K = 64
P = 128
PASS = 512          # seq cols per pass
NPASS = S // PASS
DT = D // P         # d-tiles (6)
THR = 0.05

F32 = mybir.dt.float32
BF16 = mybir.dt.bfloat16
GELU = mybir.ActivationFunctionType.Gelu

_CACHE: dict = {}


def _build(has_b2=False, act_t=None, act_s=None):
    """act_t[s] = tuple of active targets for source s (packing order);
    act_s[t] = tuple of active sources for target t (packing order)."""
    import math
    if act_t is None:
        act_t = tuple(tuple(t for t in range(H) if t != s) for s in range(H))
    if act_s is None:
        act_s = tuple(tuple(s for s in range(H) if s != t) for t in range(H))
    mchunks = [math.ceil(len(act_t[s]) * K / P) for s in range(H)]
    s2tiles = [math.ceil(len(act_s[t]) * K / P) for t in range(H)]

    nc = bacc.Bacc("TRN2", target_bir_lowering=False, debug=False)
    xtd = nc.declare_dram_parameter("xtd", [H, NPASS, P, DT, PASS], BF16, isOutput=False)
    w1h = nc.declare_dram_parameter("w1h", [H, P, DT, 3 * P], BF16, isOutput=False)
    w2h = nc.declare_dram_parameter("w2h", [H, P, 3, D], BF16, isOutput=False)
    b1h = nc.declare_dram_parameter("b1h", [P, 3, H], F32, isOutput=False)
    b2h = nc.declare_dram_parameter("b2h", [P, DT, H], F32, isOutput=False)
    outd = nc.declare_dram_parameter("outd", [H, NPASS, P, DT, PASS], BF16, isOutput=True)

    with tile.TileContext(nc) as tc:
        with tc.tile_pool(name="static", bufs=1) as st, \
             tc.tile_pool(name="xt", bufs=2) as xtp, \
             tc.tile_pool(name="ht", bufs=2) as htp, \
             tc.tile_pool(name="osb", bufs=4) as osp, \
             tc.tile_pool(name="s1_ps", bufs=2, space="PSUM") as s1p, \
             tc.tile_pool(name="s2_ps", bufs=2, space="PSUM") as s2p:

            # ---- static setup ----
            # Loads are split across the two HWDGE queues (sync, scalar) in
            # consumption order, so a consumer's queue-prefix wait covers
            # only tiles it actually needs soon.  w1[s]/xt[s] interleave at
            # pass 0 so stage-1 s=0 starts after ~1.4 MB of DMA, not 10 MB.
            # Stores go on the gpsimd SWDGE queue so they never block GELUs.
            ring = [nc.sync, nc.scalar]
            b1sb = st.tile([P, 3, H], F32, tag="b1sb")
            nc.scalar.dma_start(b1sb[:], b1h[:])
            if has_b2:
                b2sb = st.tile([P, DT, H], F32, tag="b2sb")
                nc.gpsimd.dma_start(b2sb[:], b2h[:])
            w2sb = []
            for t in range(H):
                jt = max(s2tiles[t], 1)
                w = st.tile([P, jt, D], BF16, tag=f"w2_{t}")
                nc.gpsimd.dma_start(w[:], w2h[t, :, 0:jt, :])
                w2sb.append(w)
            w1sb = [None] * H

            # ---- passes over the sequence ----
            for p in range(NPASS):
                xts = []
                for s in range(H):
                    if p == 0:
                        # startup: interleave w1/xt across both HWDGE queues
                        w = st.tile([P, DT, 3 * P], BF16, tag=f"w1_{s}",
                                    name=f"w1_{s}")
                        ring[s % 2].dma_start(w[:], w1h[s])
                        w1sb[s] = w
                    xt = xtp.tile([P, DT, PASS], BF16, tag=f"xt{s}")
                    # steady state: all loads on sync so scalar runs only GELUs
                    (ring[s % 2] if p == 0 else nc.sync).dma_start(xt[:], xtd[s, p])
                    xts.append(xt)
                hts = [htp.tile([P, 3, PASS], BF16, tag=f"ht{t}", name=f"ht{t}")
                       for t in range(H)]

                # stage 1: hT = gelu(x^T-major contractions with W1)
                for s in range(H):
                    nt = len(act_t[s])
                    for mc in range(mchunks[s]):
                        msz = min(P, nt * K - mc * P)
                        ps1 = s1p.tile([P, PASS], F32, tag="ps1")
                        for d in range(DT):
                            nc.tensor.matmul(
                                ps1[:msz, :],
                                w1sb[s][:, d, mc * P:mc * P + msz],
                                xts[s][:, d, :],
                                start=(d == 0), stop=(d == DT - 1))
                        for half in range(msz // K):
                            t = act_t[s][2 * mc + half]
                            q = act_s[t].index(s)
                            nc.scalar.activation(
                                hts[t][(q % 2) * K:(q % 2) * K + K, q // 2, :],
                                ps1[half * K:half * K + K, :],
                                GELU,
                                bias=b1sb[half * K:half * K + K, mc, s:s + 1])

                # stage 2: outT[d,n] = x^T + (gate*W2)^T @ hT, d-major.
                # 3 o-tiles share one 3-bank PSUM tile so the residual add +
                # bf16 cast is a single fused DVE op per store tile.
                for t in range(H):
                    ns = len(act_s[t])
                    for oh in range(2):          # store in 3-dtile halves
                        osb = osp.tile([P, 3, PASS], BF16, tag="osb")
                        if ns == 0:
                            nc.vector.tensor_copy(
                                osb[:], xts[t][:, oh * 3:oh * 3 + 3, :])
                        else:
                            ps2 = s2p.tile([P, 3, PASS], F32, tag="ps2")
                            for oi in range(3):
                                o = oh * 3 + oi
                                for j in range(s2tiles[t]):
                                    ksz = min(P, ns * K - j * P)
                                    nc.tensor.matmul(
                                        ps2[:, oi, :],
                                        w2sb[t][0:ksz, j, o * P:(o + 1) * P],
                                        hts[t][0:ksz, j, :],
                                        start=(j == 0), stop=(j == s2tiles[t] - 1))
                            nc.vector.tensor_add(
                                osb[:], ps2[:], xts[t][:, oh * 3:oh * 3 + 3, :])
                        if has_b2:
                            for oi in range(3):
                                nc.vector.tensor_scalar_add(
                                    osb[:, oi, :], osb[:, oi, :],
                                    b2sb[:, oh * 3 + oi, t:t + 1])
                        nc.gpsimd.dma_start(outd[t, p, :, oh * 3:oh * 3 + 3, :], osb[:])
    nc.compile()
    return nc


def prepare(inputs):
    """Host prep: gate fold + bf16 cast + layout permutes.
    Returns (in_maps, build_key)."""
    x = np.asarray(inputs["x"], dtype=np.float32)
    M = np.asarray(inputs["M"], dtype=np.float32)
    W1 = np.asarray(inputs["W1"], dtype=np.float32)
    b1 = np.asarray(inputs["b1"], dtype=np.float32)
    W2 = np.asarray(inputs["W2"], dtype=np.float32)
    b2 = np.asarray(inputs["b2"], dtype=np.float32)

    eye = np.eye(H, dtype=bool)
    gate = np.where((np.abs(M) > THR) & (~eye), M, np.zeros_like(M)).astype(np.float32)
    has_b2 = bool(np.any(b2))
    act = gate != 0.0
    act_t = tuple(tuple(int(t) for t in range(H) if act[s, t]) for s in range(H))
    act_s = tuple(tuple(int(s) for s in range(H) if act[s, t]) for t in range(H))

    # W1 columns packed per source in act_t order -> [H, 128, 6, 384] bf16
    w1f = np.zeros((H, D, 3 * P), np.float32)
    b1f = np.zeros((H, 3 * P), np.float32)
    for s in range(H):
        for i, t in enumerate(act_t[s]):
            w1f[s, :, i * K:(i + 1) * K] = W1[s, t]
            b1f[s, i * K:(i + 1) * K] = b1[s, t]
    w1h = np.ascontiguousarray(
        w1f.reshape(H, DT, P, 3 * P).transpose(0, 2, 1, 3)).astype(ml_dtypes.bfloat16)
    b1h = np.ascontiguousarray(b1f.reshape(H, 3, P).transpose(2, 1, 0))

    # gate-scaled W2 rows packed per target in act_s order -> [H, 128, 3, 768]
    w2f = np.zeros((H, 3 * P, D), np.float32)
    for t in range(H):
        for q, s in enumerate(act_s[t]):
            w2f[t, q * K:(q + 1) * K, :] = gate[s, t] * W2[s, t]
    w2h = np.ascontiguousarray(
        w2f.reshape(H, 3, P, D).transpose(0, 2, 1, 3)).astype(ml_dtypes.bfloat16)

    # gate-folded b2 per target: [128, 6, 7]
    b2f = np.einsum("st,std->td", gate, b2).astype(np.float32)   # [H, D]
    b2h = np.ascontiguousarray(b2f.reshape(H, DT, P).transpose(2, 1, 0))

    in_maps = []
    for b in range(B):
        xbf = x[:, b].astype(ml_dtypes.bfloat16)
        # [s, q, p, o, n]: element = xbf[s, q*PASS+n, o*P+p]
        xtb = np.ascontiguousarray(
            xbf.reshape(H, NPASS, PASS, DT, P).transpose(0, 1, 4, 3, 2))
        in_maps.append({
            "xtd": xtb, "w1h": w1h, "w2h": w2h, "b1h": b1h, "b2h": b2h,
        })
    return in_maps, (has_b2, act_t, act_s)


def assemble(outs):
    """Per-core outd [H, NPASS, 128, 6, 512] bf16 -> full [H, B, S, D] f32."""
    res = []
    for b in range(B):
        o = np.asarray(outs[b]["outd"])
        # out[t, q*512+n, o*128+p] = outd[t, q, p, o, n]
        res.append(o.transpose(0, 1, 4, 3, 2).reshape(H, S, D).astype(np.float32))
    return np.stack(res, axis=1)


def kernel(**inputs):
    in_maps, key = prepare(inputs)
    runner = _get_runner(key)
    outs = runner.run(in_maps)
    return assemble(outs)


class _Runner:
    """Cached PJRT executor for the SPMD bass kernel (8 cores, no donation)."""

    def __init__(self, nc):
        import jax
        from jax.sharding import Mesh, PartitionSpec, NamedSharding
        from jax.experimental.shard_map import shard_map
        from concourse import bass2jax
        bass2jax.install_neuronx_cc_hook()

        self.jax = jax
        part_name = nc.partition_id_tensor.name if nc.partition_id_tensor else None
        in_names, out_names, out_avals, zero_shapes = [], [], [], []
        for alloc in nc.m.functions[0].allocations:
            if not isinstance(alloc, mybir.MemoryLocationSet):
                continue
            name = alloc.memorylocations[0].name
            if alloc.kind == "ExternalInput":
                if name != part_name:
                    in_names.append(name)
            elif alloc.kind == "ExternalOutput":
                out_names.append(name)
                shape = tuple(alloc.tensor_shape)
                dtype = mybir.dt.np(alloc.dtype)
                out_avals.append(jax.core.ShapedArray(shape, dtype))
                zero_shapes.append((shape, dtype))
        self.n_params = len(in_names)
        self.in_names = list(in_names)
        self.out_names = out_names
        self.out_avals = out_avals
        self.zero_shapes = zero_shapes
        bind_names = tuple(in_names) + tuple(out_names)
        if part_name is not None:
            bind_names = bind_names + (part_name,)

        def _body(*args):
            operands = list(args)
            if part_name is not None:
                operands.append(bass2jax.partition_id_tensor())
            outs = bass2jax._bass_exec_p.bind(
                *operands,
                out_avals=tuple(out_avals),
                in_names=bind_names,
                out_names=tuple(out_names),
                lowering_input_output_aliases=(),
                sim_require_finite=True,
                sim_require_nnan=True,
                nc=nc,
            )
            return tuple(outs)

        devices = jax.devices()[:B]
        self.mesh = Mesh(np.asarray(devices), ("core",))
        spec = PartitionSpec("core")
        self.sharding = NamedSharding(self.mesh, spec)
        n_in = self.n_params + len(out_names)
        self.fn = jax.jit(
            shard_map(_body, mesh=self.mesh,
                      in_specs=(spec,) * n_in,
                      out_specs=(spec,) * len(out_names),
                      check_rep=False),
            keep_unused=True,
        )

    def _concat_args(self, in_maps):
        args = []
        for i, name in enumerate(self.in_names):
            args.append(np.concatenate([np.asarray(m[name]) for m in in_maps], axis=0))
        for shape, dtype in self.zero_shapes:
            args.append(np.zeros((B * shape[0],) + shape[1:], dtype))
        return args

    def run(self, in_maps):
        out_arrs = self.fn(*self._concat_args(in_maps))
        res = []
        for c in range(B):
            d = {}
            for i, name in enumerate(self.out_names):
                shape = self.out_avals[i].shape
                d[name] = np.asarray(out_arrs[i]).reshape((B,) + shape)[c]
            res.append(d)
        return res

    def benchmark(self, in_maps, iters=10):
        jax = self.jax
        args = [jax.device_put(a, self.sharding) for a in self._concat_args(in_maps)]
        outs = self.fn(*args)  # warmup / compile
        jax.block_until_ready(outs)
        import time
        t0 = time.perf_counter()
        for _ in range(iters):
            outs = self.fn(*args)
        jax.block_until_ready(outs)
        t1 = time.perf_counter()
        return (t1 - t0) / iters


def _build_from_key(key):
    has_b2, act_t, act_s = key
    return _build(has_b2=has_b2, act_t=act_t, act_s=act_s)


def _get_runner(key) -> _Runner:
    ck = ("runner", key)
    if ck not in _CACHE:
        _CACHE[ck] = _Runner(_build_from_key(key))
    return _CACHE[ck]


# revision 11
# speedup vs baseline: 1.7374x; 1.0085x over previous
"""CrossHazardInteractionLayer TRN2 kernel (v2: HBM-traffic-minimized).

Data-parallel over batch B=8 -> 8 NeuronCores (one batch element each).
Host prep: fold the |M|>thr gate into W2 (pre-scaled), cast W1/W2/x to
bf16, transpose x to feature-major (d on partitions) once.  Device:
  stage 1 per source s: hT[(t,k), n] = gelu(x[s]^T-tiles @ W1[s,:]) for
    all active targets, packed 2 targets per 128-col chunk; exact-erf
    GELU fused into the PSUM->SBUF copy on the scalar engine.
  stage 2 per target t: outT[d, n] = x[t]^T + sum over (s,k) j-tiles of
    (gate*W2)^T-stationary @ hT-moving -- output stays d-major so the
    SAME x^T tile loaded for stage 1 provides the residual (no second
    x load), fused into the PSUM->SBUF add on the vector engine (bf16
    out).  Output is written d-major bf16; host un-transposes + upcasts.
HBM traffic/core: 22 MB x + 8.3 MB weights + 22 MB out = ~52 MB
(baseline moved ~103 MB), so the kernel is tensor-engine-bound.
"""

import numpy as np
import ml_dtypes

import concourse.bass as bass
import concourse.mybir as mybir
import concourse.tile as tile
from concourse import bacc

H = 7
B = 8
S = 2048
D = 768
K = 64
P = 128
PASS = 512          # seq cols per pass
NPASS = S // PASS
DT = D // P         # d-tiles (6)
THR = 0.05

F32 = mybir.dt.float32
BF16 = mybir.dt.bfloat16
GELU = mybir.ActivationFunctionType.Gelu

_CACHE: dict = {}


def _build(has_b2=False, act_t=None, act_s=None):
    """act_t[s] = tuple of active targets for source s (packing order);
    act_s[t] = tuple of active sources for target t (packing order)."""
    import math
    if act_t is None:
        act_t = tuple(tuple(t for t in range(H) if t != s) for s in range(H))
    if act_s is None:
        act_s = tuple(tuple(s for s in range(H) if s != t) for t in range(H))
    mchunks = [math.ceil(len(act_t[s]) * K / P) for s in range(H)]
    s2tiles = [math.ceil(len(act_s[t]) * K / P) for t in range(H)]

    nc = bacc.Bacc("TRN2", target_bir_lowering=False, debug=False)
    xtd = nc.declare_dram_parameter("xtd", [H, NPASS, P, DT, PASS], BF16, isOutput=False)
    w1h = nc.declare_dram_parameter("w1h", [H, P, DT, 3 * P], BF16, isOutput=False)
    w2h = nc.declare_dram_parameter("w2h", [H, P, 3, D], BF16, isOutput=False)
    b1h = nc.declare_dram_parameter("b1h", [P, 3, H], F32, isOutput=False)
    b2h = nc.declare_dram_parameter("b2h", [P, DT, H], F32, isOutput=False)
    outd = nc.declare_dram_parameter("outd", [H, NPASS, P, DT, PASS], BF16, isOutput=True)

    with tile.TileContext(nc) as tc:
        with tc.tile_pool(name="static", bufs=1) as st, \
             tc.tile_pool(name="xt", bufs=2) as xtp, \
             tc.tile_pool(name="ht", bufs=2) as htp, \
             tc.tile_pool(name="osb", bufs=6) as osp, \
             tc.tile_pool(name="s1_ps", bufs=4, space="PSUM") as s1p, \
             tc.tile_pool(name="s2_ps", bufs=2, space="PSUM") as s2p:

            # ---- static setup ----
            # Loads are split across the two HWDGE queues (sync, scalar) in
            # consumption order, so a consumer's queue-prefix wait covers
            # only tiles it actually needs soon.  w1[s]/xt[s] interleave at
            # pass 0 so stage-1 s=0 starts after ~1.4 MB of DMA, not 10 MB.
            # Stores go on the gpsimd SWDGE queue so they never block GELUs.
            ring = [nc.sync, nc.scalar]
            b1sb = st.tile([P, 3, H], F32, tag="b1sb")
            nc.scalar.dma_start(b1sb[:], b1h[:])
            if has_b2:
                b2sb = st.tile([P, DT, H], F32, tag="b2sb")
                nc.gpsimd.dma_start(b2sb[:], b2h[:])
            w2sb = []
            for t in range(H):
                jt = max(s2tiles[t], 1)
                w = st.tile([P, jt, D], BF16, tag=f"w2_{t}")
                nc.gpsimd.dma_start(w[:], w2h[t, :, 0:jt, :])
                w2sb.append(w)
            w1sb = [None] * H

            # ---- passes over the sequence ----
            for p in range(NPASS):
                xts = []
                for s in range(H):
                    if p == 0:
                        # startup: w1[s] and xt[s] on OPPOSITE queues so the
                        # first chunks' two deps drain in parallel
                        w = st.tile([P, DT, 3 * P], BF16, tag=f"w1_{s}",
                                    name=f"w1_{s}")
                        ring[s % 2].dma_start(w[:], w1h[s])
                        w1sb[s] = w
                    xt = xtp.tile([P, DT, PASS], BF16, tag=f"xt{s}")
                    # steady state: all loads on sync so scalar runs only GELUs
                    (ring[(s + 1) % 2] if p == 0 else nc.sync).dma_start(xt[:], xtd[s, p])
                    xts.append(xt)
                hts = [htp.tile([P, 3, PASS], BF16, tag=f"ht{t}", name=f"ht{t}")
                       for t in range(H)]

                # stage 1: hT = gelu(x^T-major contractions with W1)
                for s in range(H):
                    nt = len(act_t[s])
                    for mc in range(mchunks[s]):
                        msz = min(P, nt * K - mc * P)
                        ps1 = s1p.tile([P, PASS], F32, tag="ps1")
                        for d in range(DT):
                            nc.tensor.matmul(
                                ps1[:msz, :],
                                w1sb[s][:, d, mc * P:mc * P + msz],
                                xts[s][:, d, :],
                                start=(d == 0), stop=(d == DT - 1))
                        for half in range(msz // K):
                            t = act_t[s][2 * mc + half]
                            q = act_s[t].index(s)
                            nc.scalar.activation(
                                hts[t][(q % 2) * K:(q % 2) * K + K, q // 2, :],
                                ps1[half * K:half * K + K, :],
                                GELU,
                                bias=b1sb[half * K:half * K + K, mc, s:s + 1])

                # stage 2: outT[d,n] = x^T + (gate*W2)^T @ hT, d-major.
                # 2 o-tiles share one 2-bank PSUM tile so the residual add +
                # bf16 cast is a single fused DVE op per store tile; stores
                # alternate gpsimd/sync to halve per-queue drain backlog.
                for t in range(H):
                    ns = len(act_s[t])
                    for oh in range(3):          # store in 2-dtile pairs
                        osb = osp.tile([P, 2, PASS], BF16, tag="osb")
                        if ns == 0:
                            nc.vector.tensor_copy(
                                osb[:], xts[t][:, oh * 2:oh * 2 + 2, :])
                        else:
                            ps2 = s2p.tile([P, 2, PASS], F32, tag="ps2")
                            for oi in range(2):
                                o = oh * 2 + oi
                                for j in range(s2tiles[t]):
                                    ksz = min(P, ns * K - j * P)
                                    nc.tensor.matmul(
                                        ps2[:, oi, :],
                                        w2sb[t][0:ksz, j, o * P:(o + 1) * P],
                                        hts[t][0:ksz, j, :],
                                        start=(j == 0), stop=(j == s2tiles[t] - 1))
                            nc.vector.tensor_add(
                                osb[:], ps2[:], xts[t][:, oh * 2:oh * 2 + 2, :])
                        if has_b2:
                            for oi in range(2):
                                nc.vector.tensor_scalar_add(
                                    osb[:, oi, :], osb[:, oi, :],
                                    b2sb[:, oh * 2 + oi, t:t + 1])
                        (nc.gpsimd if (t * 3 + oh) % 2 == 0 else nc.sync).dma_start(
                            outd[t, p, :, oh * 2:oh * 2 + 2, :], osb[:])
    nc.compile()
    return nc


def prepare(inputs):
    """Host prep: gate fold + bf16 cast + layout permutes.
    Returns (in_maps, build_key)."""
    x = np.asarray(inputs["x"], dtype=np.float32)
    M = np.asarray(inputs["M"], dtype=np.float32)
    W1 = np.asarray(inputs["W1"], dtype=np.float32)
    b1 = np.asarray(inputs["b1"], dtype=np.float32)
    W2 = np.asarray(inputs["W2"], dtype=np.float32)
    b2 = np.asarray(inputs["b2"], dtype=np.float32)

    eye = np.eye(H, dtype=bool)
    gate = np.where((np.abs(M) > THR) & (~eye), M, np.zeros_like(M)).astype(np.float32)
    has_b2 = bool(np.any(b2))
    act = gate != 0.0
    act_t = tuple(tuple(int(t) for t in range(H) if act[s, t]) for s in range(H))
    act_s = tuple(tuple(int(s) for s in range(H) if act[s, t]) for t in range(H))

    # W1 columns packed per source in act_t order -> [H, 128, 6, 384] bf16
    w1f = np.zeros((H, D, 3 * P), np.float32)
    b1f = np.zeros((H, 3 * P), np.float32)
    for s in range(H):
        for i, t in enumerate(act_t[s]):
            w1f[s, :, i * K:(i + 1) * K] = W1[s, t]
            b1f[s, i * K:(i + 1) * K] = b1[s, t]
    w1h = np.ascontiguousarray(
        w1f.reshape(H, DT, P, 3 * P).transpose(0, 2, 1, 3)).astype(ml_dtypes.bfloat16)
    b1h = np.ascontiguousarray(b1f.reshape(H, 3, P).transpose(2, 1, 0))

    # gate-scaled W2 rows packed per target in act_s order -> [H, 128, 3, 768]
    w2f = np.zeros((H, 3 * P, D), np.float32)
    for t in range(H):
        for q, s in enumerate(act_s[t]):
            w2f[t, q * K:(q + 1) * K, :] = gate[s, t] * W2[s, t]
    w2h = np.ascontiguousarray(
        w2f.reshape(H, 3, P, D).transpose(0, 2, 1, 3)).astype(ml_dtypes.bfloat16)

    # gate-folded b2 per target: [128, 6, 7]
    b2f = np.einsum("st,std->td", gate, b2).astype(np.float32)   # [H, D]
    b2h = np.ascontiguousarray(b2f.reshape(H, DT, P).transpose(2, 1, 0))

    in_maps = []
    for b in range(B):
        xbf = x[:, b].astype(ml_dtypes.bfloat16)
        # [s, q, p, o, n]: element = xbf[s, q*PASS+n, o*P+p]
        xtb = np.ascontiguousarray(
            xbf.reshape(H, NPASS, PASS, DT, P).transpose(0, 1, 4, 3, 2))
        in_maps.append({
            "xtd": xtb, "w1h": w1h, "w2h": w2h, "b1h": b1h, "b2h": b2h,
        })
    return in_maps, (has_b2, act_t, act_s)


def assemble(outs):
    """Per-core outd [H, NPASS, 128, 6, 512] bf16 -> full [H, B, S, D] f32."""
    res = []
    for b in range(B):
        o = np.asarray(outs[b]["outd"])
        # out[t, q*512+n, o*128+p] = outd[t, q, p, o, n]
        res.append(o.transpose(0, 1, 4, 3, 2).reshape(H, S, D).astype(np.float32))
    return np.stack(res, axis=1)


def kernel(**inputs):
    in_maps, key = prepare(inputs)
    runner = _get_runner(key)
    outs = runner.run(in_maps)
    return assemble(outs)


class _Runner:
    """Cached PJRT executor for the SPMD bass kernel (8 cores, no donation)."""

    def __init__(self, nc):
        import jax
        from jax.sharding import Mesh, PartitionSpec, NamedSharding
        from jax.experimental.shard_map import shard_map
        from concourse import bass2jax
        bass2jax.install_neuronx_cc_hook()

        self.jax = jax
        part_name = nc.partition_id_tensor.name if nc.partition_id_tensor else None
        in_names, out_names, out_avals, zero_shapes = [], [], [], []
        for alloc in nc.m.functions[0].allocations:
            if not isinstance(alloc, mybir.MemoryLocationSet):
                continue
            name = alloc.memorylocations[0].name
            if alloc.kind == "ExternalInput":
                if name != part_name:
                    in_names.append(name)
            elif alloc.kind == "ExternalOutput":
                out_names.append(name)
                shape = tuple(alloc.tensor_shape)
                dtype = mybir.dt.np(alloc.dtype)
                out_avals.append(jax.core.ShapedArray(shape, dtype))
                zero_shapes.append((shape, dtype))
        self.n_params = len(in_names)
        self.in_names = list(in_names)
        self.out_names = out_names
        self.out_avals = out_avals
        self.zero_shapes = zero_shapes
        bind_names = tuple(in_names) + tuple(out_names)
        if part_name is not None:
            bind_names = bind_names + (part_name,)

        def _body(*args):
            operands = list(args)
            if part_name is not None:
                operands.append(bass2jax.partition_id_tensor())
            outs = bass2jax._bass_exec_p.bind(
                *operands,
                out_avals=tuple(out_avals),
                in_names=bind_names,
                out_names=tuple(out_names),
                lowering_input_output_aliases=(),
                sim_require_finite=True,
                sim_require_nnan=True,
                nc=nc,
            )
            return tuple(outs)

        devices = jax.devices()[:B]
        self.mesh = Mesh(np.asarray(devices), ("core",))
        spec = PartitionSpec("core")
        self.sharding = NamedSharding(self.mesh, spec)
        n_in = self.n_params + len(out_names)
        self.fn = jax.jit(
            shard_map(_body, mesh=self.mesh,
                      in_specs=(spec,) * n_in,
                      out_specs=(spec,) * len(out_names),
                      check_rep=False),
            keep_unused=True,
        )

    def _concat_args(self, in_maps):
        args = []
        for i, name in enumerate(self.in_names):
            args.append(np.concatenate([np.asarray(m[name]) for m in in_maps], axis=0))
        for shape, dtype in self.zero_shapes:
            args.append(np.zeros((B * shape[0],) + shape[1:], dtype))
        return args

    def run(self, in_maps):
        out_arrs = self.fn(*self._concat_args(in_maps))
        res = []
        for c in range(B):
            d = {}
            for i, name in enumerate(self.out_names):
                shape = self.out_avals[i].shape
                d[name] = np.asarray(out_arrs[i]).reshape((B,) + shape)[c]
            res.append(d)
        return res

    def benchmark(self, in_maps, iters=10):
        jax = self.jax
        args = [jax.device_put(a, self.sharding) for a in self._concat_args(in_maps)]
        outs = self.fn(*args)  # warmup / compile
        jax.block_until_ready(outs)
        import time
        t0 = time.perf_counter()
        for _ in range(iters):
            outs = self.fn(*args)
        jax.block_until_ready(outs)
        t1 = time.perf_counter()
        return (t1 - t0) / iters


def _build_from_key(key):
    has_b2, act_t, act_s = key
    return _build(has_b2=has_b2, act_t=act_t, act_s=act_s)


def _get_runner(key) -> _Runner:
    ck = ("runner", key)
    if ck not in _CACHE:
        _CACHE[ck] = _Runner(_build_from_key(key))
    return _CACHE[ck]


# revision 16
# speedup vs baseline: 1.8740x; 1.0786x over previous
"""CrossHazardInteractionLayer TRN2 kernel (v2: HBM-traffic-minimized).

Data-parallel over batch B=8 -> 8 NeuronCores (one batch element each).
Host prep: fold the |M|>thr gate into W2 (pre-scaled), cast W1/W2/x to
bf16, transpose x to feature-major (d on partitions) once.  Device:
  stage 1 per source s: hT[(t,k), n] = gelu(x[s]^T-tiles @ W1[s,:]) for
    all active targets, packed 2 targets per 128-col chunk; exact-erf
    GELU fused into the PSUM->SBUF copy on the scalar engine.
  stage 2 per target t: outT[d, n] = x[t]^T + sum over (s,k) j-tiles of
    (gate*W2)^T-stationary @ hT-moving -- output stays d-major so the
    SAME x^T tile loaded for stage 1 provides the residual (no second
    x load), fused into the PSUM->SBUF add on the vector engine (bf16
    out).  Output is written d-major bf16; host un-transposes + upcasts.
HBM traffic/core: 22 MB x + 8.3 MB weights + 22 MB out = ~52 MB
(baseline moved ~103 MB), so the kernel is tensor-engine-bound.
"""

import numpy as np
import ml_dtypes

import concourse.bass as bass
import concourse.mybir as mybir
import concourse.tile as tile
from concourse import bacc

H = 7
B = 8
S = 2048
D = 768
K = 64
P = 128
PASS = 512          # seq cols per pass
NPASS = S // PASS
DT = D // P         # d-tiles (6)
THR = 0.05

F32 = mybir.dt.float32
BF16 = mybir.dt.bfloat16
GELU = mybir.ActivationFunctionType.Gelu

_CACHE: dict = {}


def _build(has_b2=False, act_t=None, act_s=None):
    """act_t[s] = tuple of active targets for source s (packing order);
    act_s[t] = tuple of active sources for target t (packing order)."""
    import math
    if act_t is None:
        act_t = tuple(tuple(t for t in range(H) if t != s) for s in range(H))
    if act_s is None:
        act_s = tuple(tuple(s for s in range(H) if s != t) for t in range(H))
    mchunks = [math.ceil(len(act_t[s]) * K / P) for s in range(H)]
    s2tiles = [math.ceil(len(act_s[t]) * K / P) for t in range(H)]

    nc = bacc.Bacc("TRN2", target_bir_lowering=False, debug=False)
    xtd = nc.declare_dram_parameter("xtd", [H, NPASS, P, DT, PASS], BF16, isOutput=False)
    w1h = nc.declare_dram_parameter("w1h", [H, P, DT, 3 * P], BF16, isOutput=False)
    w2h = nc.declare_dram_parameter("w2h", [H, P, 3, D], BF16, isOutput=False)
    b1h = nc.declare_dram_parameter("b1h", [P, 4, H], F32, isOutput=False)
    b2h = nc.declare_dram_parameter("b2h", [P, DT, H], F32, isOutput=False)
    outd = nc.declare_dram_parameter("outd", [H, NPASS, P, DT, PASS], BF16, isOutput=True)

    with tile.TileContext(nc) as tc:
        with tc.tile_pool(name="static", bufs=1) as st, \
             tc.tile_pool(name="xt", bufs=2) as xtp, \
             tc.tile_pool(name="ht", bufs=2) as htp, \
             tc.tile_pool(name="osb", bufs=6) as osp, \
             tc.tile_pool(name="s1_ps", bufs=4, space="PSUM") as s1p, \
             tc.tile_pool(name="s2_ps", bufs=2, space="PSUM") as s2p:

            # ---- static setup ----
            # Loads are split across the two HWDGE queues (sync, scalar) in
            # consumption order, so a consumer's queue-prefix wait covers
            # only tiles it actually needs soon.  w1[s]/xt[s] interleave at
            # pass 0 so stage-1 s=0 starts after ~1.4 MB of DMA, not 10 MB.
            # Stores go on the gpsimd SWDGE queue so they never block GELUs.
            ring = [nc.sync, nc.scalar]
            b1sb = st.tile([P, 4, H], F32, tag="b1sb")
            nc.scalar.dma_start(b1sb[:], b1h[:])
            w1sb = [None] * H
            w2sb = [None] * H

            # odd-tail pairing for stage 1: sources whose packed k-columns
            # end in a 64-wide tail get col-tiled two-per-PSUM-chunk
            fullc = [len(act_t[s]) * K // P for s in range(H)]
            odd = [s for s in range(H) if (len(act_t[s]) * K) % P]
            pair_after = {}
            for i in range(0, len(odd) - 1, 2):
                pair_after[odd[i + 1]] = (odd[i], odd[i + 1])
            leftover = odd[-1] if len(odd) % 2 else None

            # ---- passes over the sequence ----
            for p in range(NPASS):
                xts = []
                for s in range(H):
                    if p == 0:
                        # startup: w1[s] and xt[s] on OPPOSITE queues so the
                        # first chunks' two deps drain in parallel
                        w = st.tile([P, DT, 3 * P], BF16, tag=f"w1_{s}",
                                    name=f"w1_{s}")
                        ring[s % 2].dma_start(w[:], w1h[s])
                        w1sb[s] = w
                    xt = xtp.tile([P, DT, PASS], BF16, tag=f"xt{s}")
                    # pass 0: spread x tiles over all three queues for fast
                    # delivery; steady state: all on sync so scalar stays pure
                    if p == 0:
                        eng = nc.gpsimd if s >= 4 else ring[(s + 1) % 2]
                    else:
                        eng = nc.sync
                    eng.dma_start(xt[:], xtd[s, p])
                    xts.append(xt)
                if p == 0:
                    # w2/b2 queue behind the pass-0 x tiles on gpsimd;
                    # stage 2 first needs them ~40us in
                    if has_b2:
                        b2sb = st.tile([P, DT, H], F32, tag="b2sb")
                        nc.gpsimd.dma_start(b2sb[:], b2h[:])
                    for t in range(H):
                        jt = max(s2tiles[t], 1)
                        w = st.tile([P, jt, D], BF16, tag=f"w2_{t}", name=f"w2_{t}")
                        nc.gpsimd.dma_start(w[:], w2h[t, :, 0:jt, :])
                        w2sb[t] = w
                hts = [htp.tile([P, 3, PASS], BF16, tag=f"ht{t}", name=f"ht{t}")
                       for t in range(H)]

                # stage 1: hT = gelu(x^T-major contractions with W1).
                # Full 128-col chunks stream as before; two sources' 64-col
                # tails share one PSUM chunk via PE column tiling (the two
                # matmuls run concurrently in disjoint array column groups).
                def s1_act(ss, half, ps1, bias_slot):
                    t = act_t[ss][2 * fullc[ss]] if bias_slot >= fullc[ss] \
                        else act_t[ss][2 * bias_slot + half]
                    q = act_s[t].index(ss)
                    nc.scalar.activation(
                        hts[t][(q % 2) * K:(q % 2) * K + K, q // 2, :],
                        ps1[half * K:(half + 1) * K, :], GELU,
                        bias=b1sb[half * K:(half + 1) * K, bias_slot, ss:ss + 1])

                for s in range(H):
                    for mc in range(fullc[s]):
                        ps1 = s1p.tile([P, PASS], F32, tag="ps1")
                        for d in range(DT):
                            nc.tensor.matmul(
                                ps1[:, :],
                                w1sb[s][:, d, mc * P:(mc + 1) * P],
                                xts[s][:, d, :],
                                start=(d == 0), stop=(d == DT - 1))
                        for half in range(2):
                            s1_act(s, half, ps1, mc)
                    if s == leftover:
                        mc = fullc[s]
                        ps1 = s1p.tile([P, PASS], F32, tag="ps1")
                        for d in range(DT):
                            nc.tensor.matmul(
                                ps1[:K, :],
                                w1sb[s][:, d, mc * P:mc * P + K],
                                xts[s][:, d, :],
                                start=(d == 0), stop=(d == DT - 1))
                        s1_act(s, 0, ps1, mc)
                    if s in pair_after:
                        sA, sB = pair_after[s]
                        ps1 = s1p.tile([P, PASS], F32, tag="ps1")
                        for d in range(DT):
                            cA = fullc[sA] * P
                            nc.tensor.matmul(
                                ps1[0:K, :],
                                w1sb[sA][:, d, cA:cA + K],
                                xts[sA][:, d, :],
                                start=(d == 0), stop=(d == DT - 1),
                                tile_position=(0, 0))
                            cB = fullc[sB] * P
                            nc.tensor.matmul(
                                ps1[K:2 * K, :],
                                w1sb[sB][:, d, cB:cB + K],
                                xts[sB][:, d, :],
                                start=(d == 0), stop=(d == DT - 1),
                                tile_position=(0, K))
                        s1_act(sA, 0, ps1, fullc[sA])
                        s1_act(sB, 1, ps1, 3)

                # stage 2: outT[d,n] = x^T + (gate*W2)^T @ hT, d-major.
                # 2 o-tiles share one 2-bank PSUM tile so the residual add +
                # bf16 cast is a single fused DVE op per store tile; stores
                # alternate gpsimd/sync to halve per-queue drain backlog.
                for t in range(H):
                    ns = len(act_s[t])
                    for oh in range(3):          # store in 2-dtile pairs
                        osb = osp.tile([P, 2, PASS], BF16, tag="osb")
                        if ns == 0:
                            nc.vector.tensor_copy(
                                osb[:], xts[t][:, oh * 2:oh * 2 + 2, :])
                        else:
                            ps2 = s2p.tile([P, 2, PASS], F32, tag="ps2")
                            for oi in range(2):
                                o = oh * 2 + oi
                                for j in range(s2tiles[t]):
                                    ksz = min(P, ns * K - j * P)
                                    nc.tensor.matmul(
                                        ps2[:, oi, :],
                                        w2sb[t][0:ksz, j, o * P:(o + 1) * P],
                                        hts[t][0:ksz, j, :],
                                        start=(j == 0), stop=(j == s2tiles[t] - 1))
                            nc.vector.tensor_add(
                                osb[:], ps2[:], xts[t][:, oh * 2:oh * 2 + 2, :])
                        if has_b2:
                            for oi in range(2):
                                nc.vector.tensor_scalar_add(
                                    osb[:, oi, :], osb[:, oi, :],
                                    b2sb[:, oh * 2 + oi, t:t + 1])
                        (nc.gpsimd if (t * 3 + oh) % 2 == 0 else nc.sync).dma_start(
                            outd[t, p, :, oh * 2:oh * 2 + 2, :], osb[:])
    nc.compile()
    return nc


def prepare(inputs):
    """Host prep: gate fold + bf16 cast + layout permutes.
    Returns (in_maps, build_key)."""
    x = np.asarray(inputs["x"], dtype=np.float32)
    M = np.asarray(inputs["M"], dtype=np.float32)
    W1 = np.asarray(inputs["W1"], dtype=np.float32)
    b1 = np.asarray(inputs["b1"], dtype=np.float32)
    W2 = np.asarray(inputs["W2"], dtype=np.float32)
    b2 = np.asarray(inputs["b2"], dtype=np.float32)

    eye = np.eye(H, dtype=bool)
    gate = np.where((np.abs(M) > THR) & (~eye), M, np.zeros_like(M)).astype(np.float32)
    has_b2 = bool(np.any(b2))
    act = gate != 0.0
    act_t = tuple(tuple(int(t) for t in range(H) if act[s, t]) for s in range(H))
    act_s = tuple(tuple(int(s) for s in range(H) if act[s, t]) for t in range(H))

    # W1 columns packed per source in act_t order -> [H, 128, 6, 384] bf16
    w1f = np.zeros((H, D, 3 * P), np.float32)
    b1f = np.zeros((H, 3 * P), np.float32)
    for s in range(H):
        for i, t in enumerate(act_t[s]):
            w1f[s, :, i * K:(i + 1) * K] = W1[s, t]
            b1f[s, i * K:(i + 1) * K] = b1[s, t]
    w1h = np.ascontiguousarray(
        w1f.reshape(H, DT, P, 3 * P).transpose(0, 2, 1, 3)).astype(ml_dtypes.bfloat16)
    # slot 3: odd-tail bias replicated to both partition halves (for the
    # col-tiled paired chunks, whose second half sits at partitions 64:128)
    b1x = np.zeros((H, 4, P), np.float32)
    b1x[:, :3, :] = b1f.reshape(H, 3, P)
    for s in range(H):
        ncols = len(act_t[s]) * K
        if ncols % P:
            c0 = (ncols // P) * P
            b1x[s, 3, 0:K] = b1f[s, c0:c0 + K]
            b1x[s, 3, K:2 * K] = b1f[s, c0:c0 + K]
    b1h = np.ascontiguousarray(b1x.transpose(2, 1, 0))

    # gate-scaled W2 rows packed per target in act_s order -> [H, 128, 3, 768]
    w2f = np.zeros((H, 3 * P, D), np.float32)
    for t in range(H):
        for q, s in enumerate(act_s[t]):
            w2f[t, q * K:(q + 1) * K, :] = gate[s, t] * W2[s, t]
    w2h = np.ascontiguousarray(
        w2f.reshape(H, 3, P, D).transpose(0, 2, 1, 3)).astype(ml_dtypes.bfloat16)

    # gate-folded b2 per target: [128, 6, 7]
    b2f = np.einsum("st,std->td", gate, b2).astype(np.float32)   # [H, D]
    b2h = np.ascontiguousarray(b2f.reshape(H, DT, P).transpose(2, 1, 0))

    in_maps = []
    for b in range(B):
        xbf = x[:, b].astype(ml_dtypes.bfloat16)
        # [s, q, p, o, n]: element = xbf[s, q*PASS+n, o*P+p]
        xtb = np.ascontiguousarray(
            xbf.reshape(H, NPASS, PASS, DT, P).transpose(0, 1, 4, 3, 2))
        in_maps.append({
            "xtd": xtb, "w1h": w1h, "w2h": w2h, "b1h": b1h, "b2h": b2h,
        })
    return in_maps, (has_b2, act_t, act_s)


def assemble(outs):
    """Per-core outd [H, NPASS, 128, 6, 512] bf16 -> full [H, B, S, D] f32."""
    res = []
    for b in range(B):
        o = np.asarray(outs[b]["outd"])
        # out[t, q*512+n, o*128+p] = outd[t, q, p, o, n]
        res.append(o.transpose(0, 1, 4, 3, 2).reshape(H, S, D).astype(np.float32))
    return np.stack(res, axis=1)


def kernel(**inputs):
    in_maps, key = prepare(inputs)
    runner = _get_runner(key)
    outs = runner.run(in_maps)
    return assemble(outs)


class _Runner:
    """Cached PJRT executor for the SPMD bass kernel (8 cores, no donation)."""

    def __init__(self, nc):
        import jax
        from jax.sharding import Mesh, PartitionSpec, NamedSharding
        from jax.experimental.shard_map import shard_map
        from concourse import bass2jax
        bass2jax.install_neuronx_cc_hook()

        self.jax = jax
        part_name = nc.partition_id_tensor.name if nc.partition_id_tensor else None
        in_names, out_names, out_avals, zero_shapes = [], [], [], []
        for alloc in nc.m.functions[0].allocations:
            if not isinstance(alloc, mybir.MemoryLocationSet):
                continue
            name = alloc.memorylocations[0].name
            if alloc.kind == "ExternalInput":
                if name != part_name:
                    in_names.append(name)
            elif alloc.kind == "ExternalOutput":
                out_names.append(name)
                shape = tuple(alloc.tensor_shape)
                dtype = mybir.dt.np(alloc.dtype)
                out_avals.append(jax.core.ShapedArray(shape, dtype))
                zero_shapes.append((shape, dtype))
        self.n_params = len(in_names)
        self.in_names = list(in_names)
        self.out_names = out_names
        self.out_avals = out_avals
        self.zero_shapes = zero_shapes
        bind_names = tuple(in_names) + tuple(out_names)
        if part_name is not None:
            bind_names = bind_names + (part_name,)

        def _body(*args):
            operands = list(args)
            if part_name is not None:
                operands.append(bass2jax.partition_id_tensor())
            outs = bass2jax._bass_exec_p.bind(
                *operands,
                out_avals=tuple(out_avals),
                in_names=bind_names,
                out_names=tuple(out_names),
                lowering_input_output_aliases=(),
                sim_require_finite=True,
                sim_require_nnan=True,
                nc=nc,
            )
            return tuple(outs)

        devices = jax.devices()[:B]
        self.mesh = Mesh(np.asarray(devices), ("core",))
        spec = PartitionSpec("core")
        self.sharding = NamedSharding(self.mesh, spec)
        n_in = self.n_params + len(out_names)
        self.fn = jax.jit(
            shard_map(_body, mesh=self.mesh,
                      in_specs=(spec,) * n_in,
                      out_specs=(spec,) * len(out_names),
                      check_rep=False),
            keep_unused=True,
        )

    def _concat_args(self, in_maps):
        args = []
        for i, name in enumerate(self.in_names):
            args.append(np.concatenate([np.asarray(m[name]) for m in in_maps], axis=0))
        for shape, dtype in self.zero_shapes:
            args.append(np.zeros((B * shape[0],) + shape[1:], dtype))
        return args

    def run(self, in_maps):
        out_arrs = self.fn(*self._concat_args(in_maps))
        res = []
        for c in range(B):
            d = {}
            for i, name in enumerate(self.out_names):
                shape = self.out_avals[i].shape
                d[name] = np.asarray(out_arrs[i]).reshape((B,) + shape)[c]
            res.append(d)
        return res

    def benchmark(self, in_maps, iters=10):
        jax = self.jax
        args = [jax.device_put(a, self.sharding) for a in self._concat_args(in_maps)]
        outs = self.fn(*args)  # warmup / compile
        jax.block_until_ready(outs)
        import time
        t0 = time.perf_counter()
        for _ in range(iters):
            outs = self.fn(*args)
        jax.block_until_ready(outs)
        t1 = time.perf_counter()
        return (t1 - t0) / iters


def _build_from_key(key):
    has_b2, act_t, act_s = key
    return _build(has_b2=has_b2, act_t=act_t, act_s=act_s)


def _get_runner(key) -> _Runner:
    ck = ("runner", key)
    if ck not in _CACHE:
        _CACHE[ck] = _Runner(_build_from_key(key))
    return _CACHE[ck]
